# revision 9
# baseline (speedup 1.0000x reference)
"""Causal self-attention (B=2, T=2048, C=1024, H=16, RoPE) on 8 trn2 cores.

Sharding: core c = 4*b + g handles batch b and head group g (4 heads).
 - column-parallel W_qkv (each core computes q,k,v for its 4 heads)
 - attention fully local per (batch, head)
 - row-parallel W_proj -> partial [T, C] outputs, summed on host.

On-device layout: q,k are kept transposed [d, t]; scores are computed
transposed [tk, tq] so the causal softmax exp is a pure elementwise
PSUM->SBUF eviction (no transposes, no max subtraction -- scores for this
problem are O(10), well within fp32 exp range). Row sums come from a ones
column appended to v; normalization happens on the small yT [64, 512]
accumulators via a gpsimd partition-broadcast of 1/sum.

All matmuls run as float32r (full PE rate, ~1e-4 rounding).
"""
import numpy as np
import concourse.bass as bass
import concourse.mybir as mybir
import concourse.tile as tile
from concourse import bacc
from concourse.bass import ts, ds
from concourse.bass_utils import run_bass_kernel_spmd
from contextlib import ExitStack

F32 = mybir.dt.float32
F32R = mybir.dt.float32r
EXP = mybir.ActivationFunctionType.Exp

B, T, C, H, DH = 2, 2048, 1024, 16, 64
NCORE, G = 8, 4          # cores, head-groups
HPG = H // G             # heads per group = 4
CT = C // 128            # 8 c-tiles
TT = T // 128            # 16 t-tiles
QC = T // 512            # 4 query chunks
SCALE = 1.0 / np.sqrt(DH)
ROPE_BASE = 10000.0


def _build_nc():
    nc = bacc.Bacc("TRN2", target_bir_lowering=False, debug=False)

    xT = nc.dram_tensor("xT", [C, T], F32R, kind="ExternalInput")
    Wqk = nc.dram_tensor("Wqk", [C, 512], F32R, kind="ExternalInput")
    Wv = nc.dram_tensor("Wv", [C, 256], F32R, kind="ExternalInput")
    Wp = nc.dram_tensor("Wp", [256, C], F32R, kind="ExternalInput")
    COS2 = nc.dram_tensor("COS2", [128, T], F32, kind="ExternalInput")
    S2 = nc.dram_tensor("S2", [128, T], F32, kind="ExternalInput")
    PI = nc.dram_tensor("PI", [128, 128], F32R, kind="ExternalInput")
    TRIU = nc.dram_tensor("TRIU", [128, 128], F32, kind="ExternalInput")
    VONES = nc.dram_tensor("VONES", [128, 64], F32R, kind="ExternalInput")
    OUT = nc.dram_tensor("out", [T, C], F32, kind="ExternalOutput")

    with tile.TileContext(nc) as tc, ExitStack() as top:
        const = top.enter_context(tc.tile_pool(name="const", bufs=1))
        pi_sb = const.tile([128, 128], F32R, tag="pi")
        triu_sb = const.tile([128, 128], F32, tag="triu")
        nc.sync.dma_start(out=pi_sb[:], in_=PI[:])
        nc.sync.dma_start(out=triu_sb[:], in_=TRIU[:])

        persist = top.enter_context(tc.tile_pool(name="persist", bufs=1))
        # q/k transposed, roped, f32r: jt 0=q(h01) 1=q(h23) 2=k(h01) 3=k(h23)
        qkT = [persist.tile([128, T], F32R, tag=f"qkT{j}", name=f"qkT{j}") for j in range(4)]
        # v with ones column: [t-part, tile, head, d+1]
        v_sb = persist.tile([128, TT, HPG, DH + 1], F32R, tag="v")
        nc.sync.dma_start(
            out=v_sb[:, :, :, DH:DH + 1].rearrange("p a b c -> p (a b c)"),
            in_=VONES[:],
        )
        # normalized yT [j, t] per pair tile
        yTn = [persist.tile([128, T], F32R, tag=f"yTn{j}", name=f"yTn{j}") for j in range(2)]

        # ---------------- phase B: qkv + rope ----------------
        with ExitStack() as phb:
            bw = phb.enter_context(tc.tile_pool(name="bw", bufs=1))
            xT_sb = [bw.tile([128, T], F32R, tag=f"x{i}", name=f"x{i}") for i in range(CT)]
            wqk_sb = [bw.tile([128, 512], F32R, tag=f"wqk{i}", name=f"wqk{i}") for i in range(CT)]
            wv_sb = [bw.tile([128, 256], F32R, tag=f"wv{i}", name=f"wv{i}") for i in range(CT)]
            for i in range(CT):
                nc.sync.dma_start(out=wqk_sb[i][:], in_=Wqk[ts(i, 128), :])
                nc.sync.dma_start(out=wv_sb[i][:], in_=Wv[ts(i, 128), :])
                nc.sync.dma_start(out=xT_sb[i][:], in_=xT[ts(i, 128), :])
            rope_t = phb.enter_context(tc.tile_pool(name="rope_t", bufs=1))
            cos2_sb = rope_t.tile([128, T], F32, tag="cos2")
            s2_sb = rope_t.tile([128, T], F32, tag="s2")
            nc.sync.dma_start(out=cos2_sb[:], in_=COS2[:])
            nc.sync.dma_start(out=s2_sb[:], in_=S2[:])

            psb = phb.enter_context(tc.tile_pool(name="psb", bufs=6, space="PSUM"))
            rope_p = phb.enter_context(tc.tile_pool(name="rope_p", bufs=2))

            for tc4 in range(4):          # 512-wide t chunks
                # q,k
                ps_qk = [psb.tile([128, 512], F32, tag="pb", name=f"psqk{jj}") for jj in range(4)]
                for ct in range(CT):
                    for jt in range(4):
                        nc.tensor.matmul(
                            ps_qk[jt][:],
                            wqk_sb[ct][:, ts(jt, 128)],
                            xT_sb[ct][:, ts(tc4, 512)],
                            start=(ct == 0), stop=(ct == CT - 1),
                        )
                for jt in range(4):
                    raw = rope_p.tile([128, 512], F32R, tag="raw")
                    nc.vector.tensor_copy(raw[:], ps_qk[jt][:])
                    ps_rot = psb.tile([128, 512], F32, tag="pb")
                    nc.tensor.matmul(ps_rot[:], pi_sb[:], raw[:], start=True, stop=True)
                    t1 = rope_p.tile([128, 512], F32, tag="t1")
                    nc.vector.tensor_mul(t1[:], raw[:].bitcast(F32), cos2_sb[:, ts(tc4, 512)])
                    t2 = rope_p.tile([128, 512], F32, tag="t2")
                    nc.vector.tensor_mul(t2[:], ps_rot[:], s2_sb[:, ts(tc4, 512)])
                    nc.vector.tensor_add(qkT[jt][:, ts(tc4, 512)], t1[:], t2[:])
                # v
                for t4 in range(4):
                    tt = 4 * tc4 + t4
                    ps_v = psb.tile([128, 256], F32, tag="pb")
                    for ct in range(CT):
                        nc.tensor.matmul(
                            ps_v[:],
                            xT_sb[ct][:, ts(tt, 128)],
                            wv_sb[ct][:],
                            start=(ct == 0), stop=(ct == CT - 1),
                        )
                    nc.scalar.copy(
                        v_sb[:, tt, :, 0:DH],
                        ps_v[:].rearrange("p (h d) -> p h d", h=HPG),
                    )

        # ---------------- phase C: attention ----------------
        with ExitStack() as phc:
            wproj = phc.enter_context(tc.tile_pool(name="wproj", bufs=1))
            wp_sb = [wproj.tile([128, C], F32R, tag=f"wp{i}", name=f"wp{i}") for i in range(2)]
            for i in range(2):
                nc.sync.dma_start(out=wp_sb[i][:], in_=Wp[ts(i, 128), :])

            ps_s = phc.enter_context(tc.tile_pool(name="ps_s", bufs=2, space="PSUM"))
            ps_y = phc.enter_context(tc.tile_pool(name="ps_y", bufs=2, space="PSUM"))
            ptp = phc.enter_context(tc.tile_pool(name="ptp", bufs=3))
            smal = phc.enter_context(tc.tile_pool(name="smal", bufs=2))

            for qc in range(QC):
                for hp in range(2):
                    qT, kT = qkT[hp], qkT[2 + hp]
                    yT = [ps_y.tile([DH + 1, 512], F32, tag="yT", name=f"yT{hh}") for hh in range(2)]
                    # clean tk tiles (2 per group)
                    for grp in range(2 * qc):
                        pts = []
                        for h2 in range(2):
                            p0 = 64 * h2
                            s_ps = ps_s.tile([128, 1024], F32, tag="s")
                            for j2 in range(2):
                                tk = 256 * grp + 128 * j2
                                nc.tensor.matmul(
                                    s_ps[:, ts(j2, 512)],
                                    kT[p0:p0 + 64, ds(tk, 128)],
                                    qT[p0:p0 + 64, ts(qc, 512)],
                                    start=True, stop=True,
                                )
                            pt = ptp.tile([128, 1024], F32R, tag="pt")
                            nc.scalar.activation(pt[:], s_ps[:], EXP, scale=SCALE)
                            pts.append(pt)
                        for h2 in range(2):
                            for j2 in range(2):
                                tile_i = 2 * grp + j2
                                nc.tensor.matmul(
                                    yT[h2][:],
                                    v_sb[:, tile_i, 2 * hp + h2, :],
                                    pts[h2][:, ts(j2, 512)],
                                    start=(grp == 0 and j2 == 0), stop=False,
                                    skip_group_check=True,
                                )
                    # diagonal 512x512 block: tk tiles 4qc+j
                    for j in range(4):
                        for h2 in range(2):
                            p0 = 64 * h2
                            span = (4 - j) * 128
                            d_ps = ps_s.tile([128, 512], F32, tag="s")
                            qi = j
                            while qi < 4:
                                w = min(2, 4 - qi) * 128
                                nc.tensor.matmul(
                                    d_ps[:, ds(128 * (qi - j), w)],
                                    kT[p0:p0 + 64, ds(128 * (4 * qc + j), 128)],
                                    qT[p0:p0 + 64, ds(512 * qc + 128 * qi, w)],
                                    start=True, stop=True,
                                )
                                qi += 2
                            ptd = ptp.tile([128, 512], F32R, tag="pt")
                            nc.scalar.activation(ptd[:, 0:span], d_ps[:, 0:span], EXP, scale=SCALE)
                            nc.vector.tensor_mul(ptd[:, 0:128], ptd[:, 0:128].bitcast(F32), triu_sb[:])
                            nc.tensor.matmul(
                                yT[h2][:, ds(128 * j, span)],
                                v_sb[:, 4 * qc + j, 2 * hp + h2, :],
                                ptd[:, 0:span],
                                start=(qc == 0 and j == 0), stop=(j == 3),
                                skip_group_check=True,
                            )
                    # normalize
                    for h2 in range(2):
                        r = smal.tile([1, 512], F32, tag="r")
                        nc.vector.reciprocal(r[:], yT[h2][DH:DH + 1, :])
                        rbc = smal.tile([64, 512], F32, tag="rbc")
                        nc.gpsimd.partition_broadcast(rbc[:], r[:])
                        nc.vector.tensor_mul(
                            yTn[hp][64 * h2:64 * h2 + 64, ts(qc, 512)],
                            yT[h2][0:DH, :], rbc[:],
                        )

            # ---------------- phase D: projection ----------------
            ps_p = phc.enter_context(tc.tile_pool(name="ps_p", bufs=2, space="PSUM"))
            outp = phc.enter_context(tc.tile_pool(name="outp", bufs=3))
            for tt in range(TT):
                for half in range(2):
                    pp = ps_p.tile([128, 512], F32, tag="p")
                    for jt in range(2):
                        nc.tensor.matmul(
                            pp[:],
                            yTn[jt][:, ts(tt, 128)],
                            wp_sb[jt][:, ts(half, 512)],
                            start=(jt == 0), stop=(jt == 1),
                        )
                    ob = outp.tile([128, 512], F32, tag="ob")
                    nc.scalar.copy(ob[:], pp[:])
                    nc.sync.dma_start(out=OUT[ts(tt, 128), ds(512 * half, 512)], in_=ob[:])

    nc.finalize()
    return nc


def _rope_tables():
    inv_freq = (1.0 / (ROPE_BASE ** (np.arange(0, DH, 2, dtype=np.float32) / DH))).astype(np.float32)
    t = np.arange(T, dtype=np.float32)
    freqs = np.einsum("i,j->ij", t, inv_freq).astype(np.float32)
    emb = np.concatenate([freqs, freqs], axis=-1)          # [T, DH]
    cos = np.cos(emb).astype(np.float32)
    sin = np.sin(emb).astype(np.float32)
    cosT = cos.T.copy()                                    # [DH, T]
    sinT = sin.T.copy()
    sgn = np.ones((DH, 1), dtype=np.float32)
    sgn[0:DH // 2] = -1.0
    s2 = (sgn * sinT).astype(np.float32)
    cos2 = np.concatenate([cosT, cosT], axis=0)            # [128, T]
    s22 = np.concatenate([s2, s2], axis=0)
    return np.ascontiguousarray(cos2), np.ascontiguousarray(s22)


_NC_CACHE = None
LAST_EXEC_NS = None


def _prepare_in_maps(x, W_qkv, W_proj):
    x = np.asarray(x, dtype=np.float32)
    W_qkv = np.asarray(W_qkv, dtype=np.float32)
    W_proj = np.asarray(W_proj, dtype=np.float32)

    cos2, s2 = _rope_tables()
    pi = np.zeros((128, 128), dtype=np.float32)
    half = DH // 2
    for blk in range(2):
        for i in range(DH):
            pi[64 * blk + i, 64 * blk + (i + half) % DH] = 1.0
    triu = np.triu(np.ones((128, 128), dtype=np.float32))
    vones = np.ones((128, 64), dtype=np.float32)

    Wq, Wk, Wv_full = W_qkv[:, 0:C], W_qkv[:, C:2 * C], W_qkv[:, 2 * C:3 * C]

    in_maps = []
    for core in range(NCORE):
        b, g = core // G, core % G
        cols = slice(256 * g, 256 * g + 256)
        wqk = np.concatenate([Wq[:, cols], Wk[:, cols]], axis=1)  # [C, 512]
        in_maps.append({
            "xT": np.ascontiguousarray(x[b].T),
            "Wqk": np.ascontiguousarray(wqk),
            "Wv": np.ascontiguousarray(Wv_full[:, cols]),
            "Wp": np.ascontiguousarray(W_proj[cols, :]),
            "COS2": cos2, "S2": s2, "PI": pi, "TRIU": triu, "VONES": vones,
        })
    return in_maps


def kernel(x, W_qkv, W_proj):
    global _NC_CACHE
    if _NC_CACHE is None:
        _NC_CACHE = _build_nc()
    nc = _NC_CACHE
    in_maps = _prepare_in_maps(x, W_qkv, W_proj)

    import os
    trace = bool(int(os.environ.get("BASS_KERNEL_TRACE", "0")))
    kw = {}
    if trace:
        kw = dict(trace=True, tmpdir=os.environ.get("BASS_KERNEL_TRACE_DIR") or None)
    res = run_bass_kernel_spmd(nc, in_maps, list(range(NCORE)), **kw)
    global LAST_EXEC_NS
    LAST_EXEC_NS = res.exec_time_ns
    parts = [res.results[i]["out"].astype(np.float64) for i in range(NCORE)]
    out = np.stack([
        parts[0] + parts[1] + parts[2] + parts[3],
        parts[4] + parts[5] + parts[6] + parts[7],
    ])
    return out.astype(np.float32)


# revision 23
# speedup vs baseline: 290.5533x; 290.5533x over previous
"""Causal self-attention (B=2, T=2048, C=1024, H=16, RoPE) on 8 trn2 cores.

Sharding: core c = 4*b + g handles batch b and head group g (4 heads).
 - column-parallel W_qkv (each core computes q,k,v for its 4 heads)
 - attention fully local per (batch, head)
 - row-parallel W_proj -> partial [T, C] outputs, summed on host.

On-device layout: q,k are kept transposed [d, t]; scores are computed
transposed [tk, tq] so the causal softmax exp is a pure elementwise
PSUM->SBUF eviction (no transposes, no max subtraction -- scores for this
problem are O(10), well within fp32 exp range). Row sums come from a ones
column appended to v; normalization happens on the small yT [64, 512]
accumulators via a gpsimd partition-broadcast of 1/sum.

All matmuls run as float32r (full PE rate, ~1e-4 rounding). PSUM lives in
one shared pool (2x [128,1024] + 4x [128,512] = 8 banks) so the qkv/rope,
attention, and projection phases pipeline instead of serializing.
"""
import numpy as np
import concourse.bass as bass
import concourse.mybir as mybir
import concourse.tile as tile
from concourse import bacc
from concourse.bass import ts, ds
from concourse.bass_utils import run_bass_kernel_spmd
from contextlib import ExitStack

F32 = mybir.dt.float32
F32R = mybir.dt.float32r
EXP = mybir.ActivationFunctionType.Exp

B, T, C, H, DH = 2, 2048, 1024, 16, 64
NCORE, G = 8, 4          # cores, head-groups
HPG = H // G             # heads per group = 4
CT = C // 128            # 8 c-tiles
TT = T // 128            # 16 t-tiles
QC = T // 512            # 4 query chunks
SCALE = 1.0 / np.sqrt(DH)
ROPE_BASE = 10000.0


def _build_nc(reps=1):
    nc = bacc.Bacc("TRN2", target_bir_lowering=False, debug=False)

    xT = nc.dram_tensor("xT", [C, T], F32R, kind="ExternalInput")
    Wqk = nc.dram_tensor("Wqk", [C, 512], F32R, kind="ExternalInput")
    Wv = nc.dram_tensor("Wv", [C, 256], F32R, kind="ExternalInput")
    Wp = nc.dram_tensor("Wp", [256, C], F32R, kind="ExternalInput")
    COS2 = nc.dram_tensor("COS2", [128, T], F32, kind="ExternalInput")
    S2 = nc.dram_tensor("S2", [128, T], F32, kind="ExternalInput")
    PI = nc.dram_tensor("PI", [128, 128], F32R, kind="ExternalInput")
    TRIU = nc.dram_tensor("TRIU", [128, 128], F32, kind="ExternalInput")
    VONES = nc.dram_tensor("VONES", [128, 64], F32R, kind="ExternalInput")
    OUT = nc.dram_tensor("out", [T, C], F32, kind="ExternalOutput")

    with tile.TileContext(nc) as tc, ExitStack() as top:
        const = top.enter_context(tc.tile_pool(name="const", bufs=1))
        pi_sb = const.tile([128, 128], F32R, tag="pi")
        triu_sb = const.tile([128, 128], F32, tag="triu")
        nc.sync.dma_start(out=pi_sb[:], in_=PI[:])
        nc.sync.dma_start(out=triu_sb[:], in_=TRIU[:])

        persist = top.enter_context(tc.tile_pool(name="persist", bufs=1))
        qkT = [persist.tile([128, T], F32R, tag=f"qkT{j}", name=f"qkT{j}") for j in range(4)]
        v_sb = persist.tile([128, TT, HPG, DH + 1], F32R, tag="v")
        nc.sync.dma_start(
            out=v_sb[:, :, :, DH:DH + 1].rearrange("p a b c -> p (a b c)"),
            in_=VONES[:],
        )
        yTn = [persist.tile([128, T], F32R, tag=f"yTn{j}", name=f"yTn{j}") for j in range(2)]

        bw = top.enter_context(tc.tile_pool(name="bw", bufs=1))
        xT_sb = [bw.tile([128, T], F32R, tag=f"x{i}", name=f"x{i}") for i in range(CT)]
        wqk_sb = [bw.tile([128, 512], F32R, tag=f"wqk{i}", name=f"wqk{i}") for i in range(CT)]
        wv_sb = [bw.tile([128, 256], F32R, tag=f"wv{i}", name=f"wv{i}") for i in range(CT)]
        wproj = top.enter_context(tc.tile_pool(name="wproj", bufs=1))
        wp_sb = [wproj.tile([128, C], F32R, tag=f"wp{i}", name=f"wp{i}") for i in range(2)]
        for i in range(2):
            nc.sync.dma_start(out=wp_sb[i][:], in_=Wp[ts(i, 128), :])

        # PSUM pools: B-phase small (2 banks) + C scores big (4) + yT (2)
        psB = top.enter_context(tc.tile_pool(name="psB", bufs=2, space="PSUM"))
        psC = top.enter_context(tc.tile_pool(name="psC", bufs=2, space="PSUM"))
        psY = top.enter_context(tc.tile_pool(name="psY", bufs=2, space="PSUM"))
        rope_p = top.enter_context(tc.tile_pool(name="rope_p", bufs=2))
        cs_p = top.enter_context(tc.tile_pool(name="cs_p", bufs=1))
        ptp = top.enter_context(tc.tile_pool(name="ptp", bufs=2))
        smal = top.enter_context(tc.tile_pool(name="smal", bufs=2))
        outp = top.enter_context(tc.tile_pool(name="outp", bufs=2))

        for _rep in range(reps):
            for i in range(CT):
                nc.sync.dma_start(out=wqk_sb[i][:], in_=Wqk[ts(i, 128), :])
                nc.sync.dma_start(out=wv_sb[i][:], in_=Wv[ts(i, 128), :])
                nc.sync.dma_start(out=xT_sb[i][:], in_=xT[ts(i, 128), :])

            def b_chunk(qc):
                tc4 = qc
                cos_c = cs_p.tile([128, 512], F32, tag="cos")
                s2_c = cs_p.tile([128, 512], F32, tag="s2")
                nc.sync.dma_start(out=cos_c[:], in_=COS2[:, ts(tc4, 512)])
                nc.sync.dma_start(out=s2_c[:], in_=S2[:, ts(tc4, 512)])
                for jt in range(4):
                    ps_qk = psB.tile([128, 512], F32, tag="small", name="ps_qk")
                    for ct in range(CT):
                        nc.tensor.matmul(
                            ps_qk[:],
                            wqk_sb[ct][:, ts(jt, 128)],
                            xT_sb[ct][:, ts(tc4, 512)],
                            start=(ct == 0), stop=(ct == CT - 1),
                        )
                    raw = rope_p.tile([128, 512], F32R, tag="raw")
                    nc.vector.tensor_copy(raw[:], ps_qk[:])
                    ps_rot = psB.tile([128, 512], F32, tag="small", name="ps_rot")
                    nc.tensor.matmul(ps_rot[:], pi_sb[:], raw[:], start=True, stop=True)
                    t1 = rope_p.tile([128, 512], F32, tag="t1")
                    nc.vector.tensor_mul(t1[:], ps_qk[:], cos_c[:])
                    t2 = rope_p.tile([128, 512], F32, tag="t2")
                    nc.vector.tensor_mul(t2[:], ps_rot[:], s2_c[:])
                    nc.vector.tensor_add(qkT[jt][:, ts(tc4, 512)], t1[:], t2[:])
                for t4 in range(4):
                    tt = 4 * tc4 + t4
                    ps_v = psB.tile([128, 512], F32, tag="small", name="ps_v")
                    for ct in range(CT):
                        nc.tensor.matmul(
                            ps_v[:, 0:256],
                            xT_sb[ct][:, ts(tt, 128)],
                            wv_sb[ct][:],
                            start=(ct == 0), stop=(ct == CT - 1),
                        )
                    nc.scalar.copy(
                        v_sb[:, tt, :, 0:DH],
                        ps_v[:, 0:256].rearrange("p (h d) -> p h d", h=HPG),
                    )

            def attn(qc):
                for hp in range(2):
                    qT, kT = qkT[hp], qkT[2 + hp]
                    yT = [psY.tile([DH + 1, 512], F32, tag="yT", name=f"yT{hh}")
                          for hh in range(2)]
                    for grp in range(2 * qc):
                        pts = []
                        for h2 in range(2):
                            p0 = 64 * h2
                            s_ps = psC.tile([128, 1024], F32, tag="big", name="s_ps")
                            for j2 in range(2):
                                tk = 256 * grp + 128 * j2
                                nc.tensor.matmul(
                                    s_ps[:, ts(j2, 512)],
                                    kT[p0:p0 + 64, ds(tk, 128)],
                                    qT[p0:p0 + 64, ts(qc, 512)],
                                    start=True, stop=True,
                                )
                            pt = ptp.tile([128, 1024], F32R, tag="pt")
                            nc.scalar.activation(pt[:], s_ps[:], EXP, scale=SCALE)
                            pts.append(pt)
                        for h2 in range(2):
                            for j2 in range(2):
                                tile_i = 2 * grp + j2
                                nc.tensor.matmul(
                                    yT[h2][:],
                                    v_sb[:, tile_i, 2 * hp + h2, :],
                                    pts[h2][:, ts(j2, 512)],
                                    start=(grp == 0 and j2 == 0), stop=False,
                                    skip_group_check=True,
                                )
                    for j in range(4):
                        for h2 in range(2):
                            p0 = 64 * h2
                            span = (4 - j) * 128
                            d_ps = psC.tile([128, 1024], F32, tag="big", name="d_ps")
                            qi = j
                            while qi < 4:
                                w = min(2, 4 - qi) * 128
                                if w == 128 and qc < QC - 1:
                                    w = 256  # junk cols land past span, unused
                                nc.tensor.matmul(
                                    d_ps[:, ds(128 * (qi - j), w)],
                                    kT[p0:p0 + 64, ds(128 * (4 * qc + j), 128)],
                                    qT[p0:p0 + 64, ds(512 * qc + 128 * qi, w)],
                                    start=True, stop=True,
                                )
                                qi += 2
                            ptd = ptp.tile([128, 512], F32R, tag="pt")
                            nc.scalar.activation(ptd[:, 0:span], d_ps[:, 0:span], EXP, scale=SCALE)
                            nc.vector.tensor_mul(ptd[:, 0:128], ptd[:, 0:128].bitcast(F32), triu_sb[:])
                            nc.tensor.matmul(
                                yT[h2][:, ds(128 * j, span)],
                                v_sb[:, 4 * qc + j, 2 * hp + h2, :],
                                ptd[:, 0:span],
                                start=(qc == 0 and j == 0), stop=(j == 3),
                                skip_group_check=True,
                            )
                    for h2 in range(2):
                        r = smal.tile([1, 512], F32, tag="r", bufs=2)
                        nc.vector.reciprocal(r[:], yT[h2][DH:DH + 1, :])
                        rbc = smal.tile([64, 512], F32, tag="rbc", bufs=1)
                        nc.gpsimd.partition_broadcast(rbc[:], r[:])
                        nc.vector.tensor_mul(
                            yTn[hp][64 * h2:64 * h2 + 64, ts(qc, 512)],
                            yT[h2][0:DH, :], rbc[:],
                        )
            def proj(qc):
                for t4 in range(4):
                    tt = 4 * qc + t4
                    for half in range(2):
                        pp = psB.tile([128, 512], F32, tag="small", name="pp")
                        for jt in range(2):
                            nc.tensor.matmul(
                                pp[:],
                                yTn[jt][:, ts(tt, 128)],
                                wp_sb[jt][:, ts(half, 512)],
                                start=(jt == 0), stop=(jt == 1),
                            )
                        ob = outp.tile([128, 512], F32, tag="ob")
                        if half == 0:
                            nc.vector.tensor_copy(ob[:], pp[:])
                        else:
                            nc.scalar.copy(ob[:], pp[:])
                        nc.sync.dma_start(out=OUT[ts(tt, 128), ds(512 * half, 512)], in_=ob[:])

            b_chunk(0); b_chunk(1); attn(0); b_chunk(2); attn(1); proj(0)
            b_chunk(3); attn(2); proj(1); attn(3); proj(2); proj(3)

    nc.finalize()
    return nc


def _rope_tables():
    inv_freq = (1.0 / (ROPE_BASE ** (np.arange(0, DH, 2, dtype=np.float32) / DH))).astype(np.float32)
    t = np.arange(T, dtype=np.float32)
    freqs = np.einsum("i,j->ij", t, inv_freq).astype(np.float32)
    emb = np.concatenate([freqs, freqs], axis=-1)          # [T, DH]
    cos = np.cos(emb).astype(np.float32)
    sin = np.sin(emb).astype(np.float32)
    cosT = cos.T.copy()
    sinT = sin.T.copy()
    sgn = np.ones((DH, 1), dtype=np.float32)
    sgn[0:DH // 2] = -1.0
    s2 = (sgn * sinT).astype(np.float32)
    cos2 = np.concatenate([cosT, cosT], axis=0)            # [128, T]
    s22 = np.concatenate([s2, s2], axis=0)
    return np.ascontiguousarray(cos2), np.ascontiguousarray(s22)


_NC_CACHE = None
LAST_EXEC_NS = None


def _prepare_in_maps(x, W_qkv, W_proj):
    x = np.asarray(x, dtype=np.float32)
    W_qkv = np.asarray(W_qkv, dtype=np.float32)
    W_proj = np.asarray(W_proj, dtype=np.float32)

    cos2, s2 = _rope_tables()
    pi = np.zeros((128, 128), dtype=np.float32)
    half = DH // 2
    for blk in range(2):
        for i in range(DH):
            pi[64 * blk + i, 64 * blk + (i + half) % DH] = 1.0
    triu = np.triu(np.ones((128, 128), dtype=np.float32))
    vones = np.ones((128, 64), dtype=np.float32)

    Wq, Wk, Wv_full = W_qkv[:, 0:C], W_qkv[:, C:2 * C], W_qkv[:, 2 * C:3 * C]

    in_maps = []
    for core in range(NCORE):
        b, g = core // G, core % G
        cols = slice(256 * g, 256 * g + 256)
        wqk = np.concatenate([Wq[:, cols], Wk[:, cols]], axis=1)  # [C, 512]
        in_maps.append({
            "xT": np.ascontiguousarray(x[b].T),
            "Wqk": np.ascontiguousarray(wqk),
            "Wv": np.ascontiguousarray(Wv_full[:, cols]),
            "Wp": np.ascontiguousarray(W_proj[cols, :]),
            "COS2": cos2, "S2": s2, "PI": pi, "TRIU": triu, "VONES": vones,
        })
    return in_maps


def kernel(x, W_qkv, W_proj):
    global _NC_CACHE
    if _NC_CACHE is None:
        _NC_CACHE = _build_nc()
    nc = _NC_CACHE
    in_maps = _prepare_in_maps(x, W_qkv, W_proj)

    res = run_bass_kernel_spmd(nc, in_maps, list(range(NCORE)))
    global LAST_EXEC_NS
    LAST_EXEC_NS = res.exec_time_ns
    parts = [res.results[i]["out"].astype(np.float64) for i in range(NCORE)]
    out = np.stack([
        parts[0] + parts[1] + parts[2] + parts[3],
        parts[4] + parts[5] + parts[6] + parts[7],
    ])
    return out.astype(np.float32)


# revision 24
# speedup vs baseline: 354.2638x; 1.2193x over previous
"""Causal self-attention (B=2, T=2048, C=1024, H=16, RoPE) on 8 trn2 cores.

Sharding: core c = 4*b + g handles batch b and head group g (4 heads).
 - column-parallel W_qkv (each core computes q,k,v for its 4 heads)
 - attention fully local per (batch, head)
 - row-parallel W_proj -> partial [T, C] outputs, summed on host.

On-device layout: q,k are kept transposed [d, t]; scores are computed
transposed [tk, tq] so the causal softmax exp is a pure elementwise
PSUM->SBUF eviction (no transposes, no max subtraction -- scores for this
problem are O(10), well within fp32 exp range). Row sums come from a ones
column appended to v; normalization happens on the small yT [64, 512]
accumulators via a gpsimd partition-broadcast of 1/sum.

All matmuls run as float32r (full PE rate, ~1e-4 rounding). PSUM lives in
one shared pool (2x [128,1024] + 4x [128,512] = 8 banks) so the qkv/rope,
attention, and projection phases pipeline instead of serializing.
"""
import numpy as np
import concourse.bass as bass
import concourse.mybir as mybir
import concourse.tile as tile
from concourse import bacc
from concourse.bass import ts, ds
from concourse.bass_utils import run_bass_kernel_spmd
from contextlib import ExitStack

F32 = mybir.dt.float32
F32R = mybir.dt.float32r
EXP = mybir.ActivationFunctionType.Exp

B, T, C, H, DH = 2, 2048, 1024, 16, 64
NCORE, G = 8, 4          # cores, head-groups
HPG = H // G             # heads per group = 4
CT = C // 128            # 8 c-tiles
TT = T // 128            # 16 t-tiles
QC = T // 512            # 4 query chunks
SCALE = 1.0 / np.sqrt(DH)
ROPE_BASE = 10000.0


def _build_nc(reps=1):
    nc = bacc.Bacc("TRN2", target_bir_lowering=False, debug=False)

    xT = nc.dram_tensor("xT", [C, T], F32R, kind="ExternalInput")
    Wqk = nc.dram_tensor("Wqk", [C, 512], F32R, kind="ExternalInput")
    Wv = nc.dram_tensor("Wv", [C, 256], F32R, kind="ExternalInput")
    Wp = nc.dram_tensor("Wp", [256, C], F32R, kind="ExternalInput")
    COS2 = nc.dram_tensor("COS2", [128, T], F32, kind="ExternalInput")
    S2 = nc.dram_tensor("S2", [128, T], F32, kind="ExternalInput")
    PI = nc.dram_tensor("PI", [128, 128], F32R, kind="ExternalInput")
    TRIU = nc.dram_tensor("TRIU", [128, 128], F32, kind="ExternalInput")
    VONES = nc.dram_tensor("VONES", [128, 64], F32R, kind="ExternalInput")
    OUT = nc.dram_tensor("out", [T, C], F32, kind="ExternalOutput")

    with tile.TileContext(nc) as tc, ExitStack() as top:
        const = top.enter_context(tc.tile_pool(name="const", bufs=1))
        pi_sb = const.tile([128, 128], F32R, tag="pi")
        triu_sb = const.tile([128, 128], F32, tag="triu")
        nc.sync.dma_start(out=pi_sb[:], in_=PI[:])
        nc.sync.dma_start(out=triu_sb[:], in_=TRIU[:])

        persist = top.enter_context(tc.tile_pool(name="persist", bufs=1))
        qkT = [persist.tile([128, T], F32R, tag=f"qkT{j}", name=f"qkT{j}") for j in range(4)]
        v_sb = persist.tile([128, TT, HPG, DH + 1], F32R, tag="v")
        nc.sync.dma_start(
            out=v_sb[:, :, :, DH:DH + 1].rearrange("p a b c -> p (a b c)"),
            in_=VONES[:],
        )
        yTn = [persist.tile([128, T], F32R, tag=f"yTn{j}", name=f"yTn{j}") for j in range(2)]

        bw = top.enter_context(tc.tile_pool(name="bw", bufs=1))
        xT_sb = [bw.tile([128, T], F32R, tag=f"x{i}", name=f"x{i}") for i in range(CT)]
        wqk_sb = [bw.tile([128, 512], F32R, tag=f"wqk{i}", name=f"wqk{i}") for i in range(CT)]
        wv_sb = [bw.tile([128, 256], F32R, tag=f"wv{i}", name=f"wv{i}") for i in range(CT)]
        wproj = top.enter_context(tc.tile_pool(name="wproj", bufs=1))
        wp_sb = [wproj.tile([128, C], F32R, tag=f"wp{i}", name=f"wp{i}") for i in range(2)]
        for i in range(2):
            nc.sync.dma_start(out=wp_sb[i][:], in_=Wp[ts(i, 128), :])

        # PSUM pools: B-phase small (2 banks) + C scores big (4) + yT (2)
        psB = top.enter_context(tc.tile_pool(name="psB", bufs=2, space="PSUM"))
        psC = top.enter_context(tc.tile_pool(name="psC", bufs=2, space="PSUM"))
        psY = top.enter_context(tc.tile_pool(name="psY", bufs=2, space="PSUM"))
        rope_p = top.enter_context(tc.tile_pool(name="rope_p", bufs=2))
        cs_p = top.enter_context(tc.tile_pool(name="cs_p", bufs=1))
        ptp = top.enter_context(tc.tile_pool(name="ptp", bufs=2))
        smal = top.enter_context(tc.tile_pool(name="smal", bufs=2))
        outp = top.enter_context(tc.tile_pool(name="outp", bufs=2))

        for _rep in range(reps):
            for i in range(CT):
                nc.sync.dma_start(out=wqk_sb[i][:], in_=Wqk[ts(i, 128), :])
                nc.sync.dma_start(out=wv_sb[i][:], in_=Wv[ts(i, 128), :])
                nc.sync.dma_start(out=xT_sb[i][:], in_=xT[ts(i, 128), :])

            def b_chunk(qc):
                tc4 = qc
                cos_c = cs_p.tile([128, 512], F32, tag="cos")
                s2_c = cs_p.tile([128, 512], F32, tag="s2")
                nc.sync.dma_start(out=cos_c[:], in_=COS2[:, ts(tc4, 512)])
                nc.sync.dma_start(out=s2_c[:], in_=S2[:, ts(tc4, 512)])
                for jt in range(4):
                    ps_qk = psB.tile([128, 512], F32, tag="small", name="ps_qk")
                    for ct in range(CT):
                        nc.tensor.matmul(
                            ps_qk[:],
                            wqk_sb[ct][:, ts(jt, 128)],
                            xT_sb[ct][:, ts(tc4, 512)],
                            start=(ct == 0), stop=(ct == CT - 1),
                        )
                    raw = rope_p.tile([128, 512], F32R, tag="raw")
                    nc.vector.tensor_copy(raw[:], ps_qk[:])
                    ps_rot = psB.tile([128, 512], F32, tag="small", name="ps_rot")
                    nc.tensor.matmul(ps_rot[:], pi_sb[:], raw[:], start=True, stop=True)
                    t1 = rope_p.tile([128, 512], F32, tag="t1")
                    nc.vector.tensor_mul(t1[:], ps_qk[:], cos_c[:])
                    t2 = rope_p.tile([128, 512], F32, tag="t2")
                    nc.vector.tensor_mul(t2[:], ps_rot[:], s2_c[:])
                    nc.vector.tensor_add(qkT[jt][:, ts(tc4, 512)], t1[:], t2[:])
                for t4 in range(4):
                    tt = 4 * tc4 + t4
                    ps_v = psB.tile([128, 512], F32, tag="small", name="ps_v")
                    for ct in range(CT):
                        nc.tensor.matmul(
                            ps_v[:, 0:256],
                            xT_sb[ct][:, ts(tt, 128)],
                            wv_sb[ct][:],
                            start=(ct == 0), stop=(ct == CT - 1),
                        )
                    nc.scalar.copy(
                        v_sb[:, tt, :, 0:DH],
                        ps_v[:, 0:256].rearrange("p (h d) -> p h d", h=HPG),
                    )

            def attn(qc):
                for hp in range(2):
                    qT, kT = qkT[hp], qkT[2 + hp]
                    yT = [psY.tile([DH + 1, 512], F32, tag="yT", name=f"yT{hh}")
                          for hh in range(2)]
                    for grp in range(2 * qc):
                        pts = []
                        for h2 in range(2):
                            p0 = 64 * h2
                            s_ps = psC.tile([128, 1024], F32, tag="big", name="s_ps")
                            for j2 in range(2):
                                tk = 256 * grp + 128 * j2
                                nc.tensor.matmul(
                                    s_ps[:, ts(j2, 512)],
                                    kT[p0:p0 + 64, ds(tk, 128)],
                                    qT[p0:p0 + 64, ts(qc, 512)],
                                    start=True, stop=True,
                                )
                            pt = ptp.tile([128, 1024], F32R, tag="pt")
                            nc.scalar.activation(pt[:], s_ps[:], EXP, scale=SCALE)
                            pts.append(pt)
                        for h2 in range(2):
                            for j2 in range(2):
                                tile_i = 2 * grp + j2
                                nc.tensor.matmul(
                                    yT[h2][:],
                                    v_sb[:, tile_i, 2 * hp + h2, :],
                                    pts[h2][:, ts(j2, 512)],
                                    start=(grp == 0 and j2 == 0), stop=False,
                                    skip_group_check=True,
                                )
                    for jg, js in ((0, (0, 1)), (1, (2, 3))):
                        for h2 in range(2):
                            p0 = 64 * h2
                            d_ps = psC.tile([128, 1024], F32, tag="big", name="d_ps")
                            offs = {}
                            off = 0
                            for j in js:
                                offs[j] = off
                                span = (4 - j) * 128
                                qi = j
                                while qi < 4:
                                    w = min(2, 4 - qi) * 128
                                    if w == 128 and qc < QC - 1:
                                        w = 256  # junk cols land past span, unused
                                    nc.tensor.matmul(
                                        d_ps[:, ds(off + 128 * (qi - j), w)],
                                        kT[p0:p0 + 64, ds(128 * (4 * qc + j), 128)],
                                        qT[p0:p0 + 64, ds(512 * qc + 128 * qi, w)],
                                        start=True, stop=True,
                                    )
                                    qi += 2
                                off += span
                            ptd = ptp.tile([128, 1024], F32R, tag="pt")
                            nc.scalar.activation(ptd[:, 0:off], d_ps[:, 0:off], EXP, scale=SCALE)
                            for j in js:
                                o = offs[j]
                                nc.vector.tensor_mul(
                                    ptd[:, ds(o, 128)], ptd[:, ds(o, 128)].bitcast(F32), triu_sb[:])
                            for j in js:
                                o = offs[j]
                                span = (4 - j) * 128
                                nc.tensor.matmul(
                                    yT[h2][:, ds(128 * j, span)],
                                    v_sb[:, 4 * qc + j, 2 * hp + h2, :],
                                    ptd[:, ds(o, span)],
                                    start=(qc == 0 and j == 0), stop=(j == 3),
                                    skip_group_check=True,
                                )
                    for h2 in range(2):
                        r = smal.tile([1, 512], F32, tag="r", bufs=2)
                        nc.vector.reciprocal(r[:], yT[h2][DH:DH + 1, :])
                        rbc = smal.tile([64, 512], F32, tag="rbc", bufs=1)
                        nc.gpsimd.partition_broadcast(rbc[:], r[:])
                        nc.vector.tensor_mul(
                            yTn[hp][64 * h2:64 * h2 + 64, ts(qc, 512)],
                            yT[h2][0:DH, :], rbc[:],
                        )
            def proj(qc):
                for t4 in range(4):
                    tt = 4 * qc + t4
                    for half in range(2):
                        pp = psB.tile([128, 512], F32, tag="small", name="pp")
                        for jt in range(2):
                            nc.tensor.matmul(
                                pp[:],
                                yTn[jt][:, ts(tt, 128)],
                                wp_sb[jt][:, ts(half, 512)],
                                start=(jt == 0), stop=(jt == 1),
                            )
                        ob = outp.tile([128, 512], F32, tag="ob")
                        if half == 0:
                            nc.vector.tensor_copy(ob[:], pp[:])
                        else:
                            nc.scalar.copy(ob[:], pp[:])
                        nc.sync.dma_start(out=OUT[ts(tt, 128), ds(512 * half, 512)], in_=ob[:])

            b_chunk(0); b_chunk(1); attn(0); b_chunk(2); attn(1); proj(0)
            b_chunk(3); attn(2); proj(1); attn(3); proj(2); proj(3)

    nc.finalize()
    return nc


def _rope_tables():
    inv_freq = (1.0 / (ROPE_BASE ** (np.arange(0, DH, 2, dtype=np.float32) / DH))).astype(np.float32)
    t = np.arange(T, dtype=np.float32)
    freqs = np.einsum("i,j->ij", t, inv_freq).astype(np.float32)
    emb = np.concatenate([freqs, freqs], axis=-1)          # [T, DH]
    cos = np.cos(emb).astype(np.float32)
    sin = np.sin(emb).astype(np.float32)
    cosT = cos.T.copy()
    sinT = sin.T.copy()
    sgn = np.ones((DH, 1), dtype=np.float32)
    sgn[0:DH // 2] = -1.0
    s2 = (sgn * sinT).astype(np.float32)
    cos2 = np.concatenate([cosT, cosT], axis=0)            # [128, T]
    s22 = np.concatenate([s2, s2], axis=0)
    return np.ascontiguousarray(cos2), np.ascontiguousarray(s22)


_NC_CACHE = None
LAST_EXEC_NS = None


def _prepare_in_maps(x, W_qkv, W_proj):
    x = np.asarray(x, dtype=np.float32)
    W_qkv = np.asarray(W_qkv, dtype=np.float32)
    W_proj = np.asarray(W_proj, dtype=np.float32)

    cos2, s2 = _rope_tables()
    pi = np.zeros((128, 128), dtype=np.float32)
    half = DH // 2
    for blk in range(2):
        for i in range(DH):
            pi[64 * blk + i, 64 * blk + (i + half) % DH] = 1.0
    triu = np.triu(np.ones((128, 128), dtype=np.float32))
    vones = np.ones((128, 64), dtype=np.float32)

    Wq, Wk, Wv_full = W_qkv[:, 0:C], W_qkv[:, C:2 * C], W_qkv[:, 2 * C:3 * C]

    in_maps = []
    for core in range(NCORE):
        b, g = core // G, core % G
        cols = slice(256 * g, 256 * g + 256)
        wqk = np.concatenate([Wq[:, cols], Wk[:, cols]], axis=1)  # [C, 512]
        in_maps.append({
            "xT": np.ascontiguousarray(x[b].T),
            "Wqk": np.ascontiguousarray(wqk),
            "Wv": np.ascontiguousarray(Wv_full[:, cols]),
            "Wp": np.ascontiguousarray(W_proj[cols, :]),
            "COS2": cos2, "S2": s2, "PI": pi, "TRIU": triu, "VONES": vones,
        })
    return in_maps


def kernel(x, W_qkv, W_proj):
    global _NC_CACHE
    if _NC_CACHE is None:
        _NC_CACHE = _build_nc()
    nc = _NC_CACHE
    in_maps = _prepare_in_maps(x, W_qkv, W_proj)

    res = run_bass_kernel_spmd(nc, in_maps, list(range(NCORE)))
    global LAST_EXEC_NS
    LAST_EXEC_NS = res.exec_time_ns
    parts = [res.results[i]["out"].astype(np.float64) for i in range(NCORE)]
    out = np.stack([
        parts[0] + parts[1] + parts[2] + parts[3],
        parts[4] + parts[5] + parts[6] + parts[7],
    ])
    return out.astype(np.float32)
